# revision 2
# baseline (speedup 1.0000x reference)
"""Correlation layer (avgpool2x2 + all-pairs view correlation) for Trainium2.

Reference computation (hardcoded shapes):
  x: (6, 512, 90, 90) fp32, n=3 views, b=2 samples.
  xp = avgpool2x2(x)                      -> (6, 512, 45, 45)
  xf = xp.reshape(2, 3, 512, 2025)
  for each sample, for the 6 ordered view pairs (i, j), i != j:
      corr[k, q, p] = sum_c xf[i, c, q] * xf[j, c, p]
  out: (12, 2025, 45, 45) fp32

Sharding over 8 cores: core = (b, s) with sample b in {0,1} and q-stripe
s in {0..3}.  The 2025 pooled pixels are treated as an unordered set: the
host delivers each core's raw data as 2048 pooling quads (2x2 raw blocks)
in pixel order rotated left by 512*s (mod 2025, so 23 quads appear twice).
The device pools them in that order, computes all 6 ordered pairs for
q-rows [0:512) of its rotated pixel space (= original pixels
[512s : 512s+512) mod 2025) against the full p range [0:2025) (rotated),
and the host un-rotates the p axis of the output.

Each core: DMA in 3 views x 4 channel-groups as fp16 quads (25.2 MB),
avg-pool on DVE (quad pair-add + pair-reduce) into bf16 features
F[v][g] = [128ch, 2048pix], run 6 pairs x 4 q-tiles x 4 cgroups matmuls
on PE (bf16, full 128-row tiles, no padding waste), scale by 1/16 during
PSUM->SBUF eviction (alternating ACT/DVE), DMA out fp16 (12.4 MB) on the
ACT HWDGE ring so input (SP ring) and output FIFOs stay independent.
"""

import numpy as np

_NC = None

# Ordered pairs in reference k-order; emitted so view-2-dependent pairs
# come last (views DMA in order 0,1,2 -> pairs (0,1),(1,0) can start
# before view 2 has arrived).
_PAIRS = [(0, 0, 1), (2, 1, 0), (1, 0, 2), (4, 2, 0), (3, 1, 2), (5, 2, 1)]

_QT = 4              # q tiles of 128 per pair per core (512 q-rows)
_NPIX = 2025
_NBLK = 2048         # quads per (view, cgroup) chunk incl. 23 wrapped dups
_NCHUNK = [512, 512, 512, 489]  # moving-dim chunks covering 2025


def _build_nc(reps=None, unroll=1, ablate=(), evsplit=True):
    """Build the per-core program.  reps: if set, wrap the body in an
    on-device For_i loop executing it `reps` times total (used only for
    timing); `unroll` bodies are emitted per loop iteration."""
    from contextlib import nullcontext

    from concourse import bacc
    import concourse.mybir as mybir
    from concourse.tile import TileContext

    f32 = mybir.dt.float32
    f16 = mybir.dt.float16
    bf16 = mybir.dt.bfloat16

    nc = bacc.Bacc("TRN2", target_bir_lowering=False, debug=False, num_devices=8)
    x = nc.dram_tensor("x", (3, 4, 128, 4 * _NBLK), f16, kind="ExternalInput")
    out = nc.dram_tensor("out", (6, _QT * 128, _NPIX), f16, kind="ExternalOutput")

    if reps is not None:
        assert reps % unroll == 0, (reps, unroll)
        n_iter = reps // unroll
    else:
        unroll = 1

    with TileContext(nc) as tc:
        with (
            tc.tile_pool(name="fpool", bufs=min(2, max(unroll, 1)) if reps else 1) as fpool,
            tc.tile_pool(name="stage", bufs=3) as stage,
            tc.tile_pool(name="t1p", bufs=2) as t1p,
            tc.tile_pool(name="opool", bufs=2) as opool,
            tc.tile_pool(name="psum", bufs=2, space="PSUM") as psum,
        ):
            loop = (
                tc.For_i(
                    0, n_iter, 1,
                    hint_engines=(
                        mybir.EngineType.PE,
                        mybir.EngineType.SP,
                        mybir.EngineType.Activation,
                        mybir.EngineType.DVE,
                    ),
                )
                if reps is not None
                else nullcontext()
            )
            with loop:
                for _u in range(unroll):
                    # Pooled features, bf16 for full-rate PE matmul.
                    F = [
                        [fpool.tile([128, _NBLK], bf16, tag=f"F_{v}_{g}", name=f"F_{v}_{g}") for g in range(4)]
                        for v in range(3)
                    ]
                    # --- avg-pool 2x2 (sums; /16 applied at eviction) ---
                    for v in range(3):
                        for g in range(4):
                            raw = stage.tile([128, 4 * _NBLK], f16, tag="raw", name="raw")
                            nc.sync.dma_start(raw[:], x[v, g])
                            if "pool" in ablate:
                                continue
                            rv = raw[:].rearrange("p (a two) -> p a two", two=2)
                            t1 = t1p.tile([128, 2 * _NBLK], f16, tag="t1", name="t1")
                            nc.vector.tensor_tensor(
                                out=t1[:],
                                in0=rv[:, :, 0],
                                in1=rv[:, :, 1],
                                op=mybir.AluOpType.add,
                            )
                            with nc.allow_low_precision(reason="bf16 pooled features"):
                                nc.vector.reduce_sum(
                                    out=F[v][g][:],
                                    in_=t1[:].rearrange("p (a two) -> p a two", two=2),
                                    axis=mybir.AxisListType.X,
                                )

                    if "pool" in ablate:
                        for v in range(3):
                            for g in range(4):
                                nc.vector.memset(F[v][g][:].bitcast(f16), 0.0)

                    # --- correlation matmuls ---
                    for pi, a, b in _PAIRS:
                        ot = opool.tile([128, _QT, _NPIX], f16, tag="ot", name="ot")
                        for qt in range(_QT):
                            q0 = qt * 128
                            pt = psum.tile([128, _NPIX], f32, tag="pt", name="pt")
                            if "mm" in ablate:
                                if "evict" not in ablate:
                                    nc.scalar.mul(
                                        ot[:, qt, :], F[a][0][:, :_NPIX].bitcast(f16), 1.0
                                    )
                                continue
                            for g in range(4):
                                n0 = 0
                                for ns in _NCHUNK:
                                    nc.tensor.matmul(
                                        pt[:, n0 : n0 + ns],
                                        lhsT=F[a][g][:, q0 : q0 + 128],
                                        rhs=F[b][g][:, n0 : n0 + ns],
                                        start=(g == 0),
                                        stop=(g == 3),
                                    )
                                    n0 += ns
                            if "evict" not in ablate:
                                if evsplit and qt % 2:
                                    nc.vector.tensor_scalar_mul(
                                        ot[:, qt, :], pt[:], 1.0 / 16.0
                                    )
                                else:
                                    nc.scalar.mul(ot[:, qt, :], pt[:], 1.0 / 16.0)
                        if "out" in ablate:
                            continue
                        # One 2.07 MB store per pair.
                        nc.scalar.dma_start(
                            out[pi].rearrange("(t p) s -> p t s", p=128), ot[:]
                        )

    nc.finalize()
    return nc


def _core_inputs(x):
    """Per-core raw input: (3, 4, 128, 8192) fp16 pooling quads, pixel
    order rotated left by 512*s."""
    x = np.asarray(x, dtype=np.float16)
    # (6, 512, 90, 90) -> (6, 512, 2025 quads, 4)
    quads = (
        x.reshape(6, 512, 45, 2, 45, 2)
        .transpose(0, 1, 2, 4, 3, 5)
        .reshape(6, 512, 2025, 4)
    )
    ins = []
    for c in range(8):
        b, s = c // 4, c % 4
        idx = (np.arange(_NBLK) + 512 * s) % _NPIX
        xb = quads[b * 3 : (b + 1) * 3][:, :, idx]
        ins.append({"x": np.ascontiguousarray(xb).reshape(3, 4, 128, 4 * _NBLK)})
    return ins


def _gather(results):
    """Assemble the 8 per-core outputs into the full (12, 2025, 45, 45)."""
    out = np.empty((12, _NPIX, _NPIX), dtype=np.float32)
    for c in range(8):
        b, s = c // 4, c % 4
        oc = results[c]["out"].astype(np.float32)  # (6, 512, 2025) rotated
        oc = np.roll(oc, 512 * s, axis=2)  # un-rotate p axis
        rows = (512 * s + np.arange(512)) % _NPIX
        n_contig = _NPIX - 512 * s
        for k in range(6):
            if n_contig >= 512:
                out[b * 6 + k, 512 * s : 512 * s + 512] = oc[k]
            else:
                out[b * 6 + k, 512 * s :] = oc[k, :n_contig]
                out[b * 6 + k, : 512 - n_contig] = oc[k, n_contig:]
    return out.reshape(12, _NPIX, 45, 45)


def kernel(x, n):
    global _NC
    x = np.asarray(x, dtype=np.float32)
    assert int(n) == 3 and x.shape == (6, 512, 90, 90), (x.shape, n)
    from concourse.bass_utils import run_bass_kernel_spmd

    if _NC is None:
        _NC = _build_nc()
    res = run_bass_kernel_spmd(_NC, _core_inputs(x), core_ids=list(range(8)))
    return _gather(res.results)
